# revision 15
# baseline (speedup 1.0000x reference)
"""Trainium2 Bass kernel for nn_AnatomicalRelationshipModule.

Data-parallel over 8 NeuronCores (8 images each). Per core:
  - dense per-box quantities + PE transposes to row-major planes
  - pairwise IoU>0.5 test (exact fp32 replica of reference math) with
    PE matmul OR-reduction against arytenoid flags
  - candidate mask, stream-compaction of candidate rows (sparse_gather)
  - MLP refinement only on gathered candidate rows (dma_gather, bf16 PE)
  - output assembled by dma_scatter_add onto a zeroed canvas
"""
import sys
import numpy as np

sys.path.insert(0, '/opt/trn_rl_repo')

import concourse.bass as bass
import concourse.bacc as bacc
import concourse.mybir as mybir
import concourse.tile as tile
from concourse import library_config
from concourse.bass import _add_dep_helper
from concourse.bass_utils import run_bass_kernel_spmd
from contextlib import ExitStack
import ml_dtypes

f32 = mybir.dt.float32
bf16 = mybir.dt.bfloat16
i16 = mybir.dt.int16
u32 = mybir.dt.uint32
Alu = mybir.AluOpType
Act = mybir.ActivationFunctionType

B_LOC = 8
Q = 1000
D = 256
THR = np.float32(0.84729785)   # fp32 boundary of sigmoid(x) > 0.7
CANDP = 256
NROW = 8064
EW = 64

_CACHED = {}


def build():
    nc = bacc.Bacc("TRN2", target_bir_lowering=False, debug=False)
    boxlog = nc.dram_tensor("boxlog", [NROW, EW], f32, kind="ExternalInput").ap()
    featbf = nc.dram_tensor("featbf", [8001, D], bf16, kind="ExternalInput").ap()
    wp1 = nc.dram_tensor("wp1", [D, D], bf16, kind="ExternalInput").ap()
    wp2 = nc.dram_tensor("wp2", [D, D], bf16, kind="ExternalInput").ap()
    wb1 = nc.dram_tensor("wb1", [D, D], bf16, kind="ExternalInput").ap()
    wb2 = nc.dram_tensor("wb2", [D, D], bf16, kind="ExternalInput").ap()
    whd = nc.dram_tensor("whd", [D, 8], bf16, kind="ExternalInput").ap()
    cvecT = nc.dram_tensor("cvecT", [D, 8], f32, kind="ExternalInput").ap()
    iotap1w = nc.dram_tensor("iotap1w", [16, 504], f32, kind="ExternalInput").ap()
    iotaw = nc.dram_tensor("iotaw", [16, 16], f32, kind="ExternalInput").ap()
    identd = nc.dram_tensor("identd", [128, 128], f32, kind="ExternalInput").ap()

    canvas = nc.dram_tensor("canvas", [8192, EW], f32, kind="ExternalOutput").ap()
    bounce = nc.dram_tensor("bounce", [NROW, EW], f32, kind="ExternalOutput").ap()
    bounce2 = nc.dram_tensor("bounce2", [5, NROW], f32).ap()

    seq = [None, []]

    def sdep(binst):
        if seq[0] is not None:
            _add_dep_helper(binst.ins, seq[0].ins, True, "lib dep")
        seq[1].append(binst)
        return binst

    def sload(lib):
        b = nc.gpsimd.load_library(lib)
        for p in seq[1]:
            _add_dep_helper(b.ins, p.ins, True, "lib order")
        seq[0] = b
        seq[1] = []
        return b

    with tile.TileContext(nc) as tc, ExitStack() as ctx:
        pc = ctx.enter_context(tc.tile_pool(name="const", bufs=1))
        pd = ctx.enter_context(tc.tile_pool(name="dense", bufs=1))
        pw = ctx.enter_context(tc.tile_pool(name="work", bufs=2))
        psc = ctx.enter_context(tc.tile_pool(name="scratch", bufs=1))
        ps_ = ctx.enter_context(tc.tile_pool(name="small", bufs=2))
        pp = ctx.enter_context(tc.tile_pool(name="psum", bufs=1, space="PSUM"))
        pp2 = ctx.enter_context(tc.tile_pool(name="psum2", bufs=2, space="PSUM"))
        pb = ctx.enter_context(tc.tile_pool(name="big", bufs=1))

        # ------- S0: zero canvas -------
        idn = pc.tile([128, 128], f32)
        nc.sync.dma_start(idn[:], identd[:, :])
        zsb = pc.tile([128, 512], f32)
        nc.vector.memset(zsb[:], 0.0)
        for c in range(8):
            nc.sync.dma_start(canvas[1024 * c:1024 * (c + 1), :], zsb[:])

        # ------- S1: dense load -------
        # cols: 0 cx 1 cy 2 w 3 h 4..9 logits | 10 x1 11 x2 12 y1 13 y2 14 area
        #       15 lf 16 rf 17 la 18 ra 20 halfw 21 halfh 30 cl 31 cr
        dense = pd.tile([128, EW, EW], f32)
        nc.vector.memset(dense[:], 0.0)
        slot_rows = []
        for b in range(B_LOC):
            for jt in range(8):
                s = 8 * b + jt
                r0 = 1000 * b + 128 * jt
                n = 104 if jt == 7 else 128
                slot_rows.append((s, r0, n))
                nc.sync.dma_start(dense[0:n, s, :], boxlog[r0:r0 + n, :])

        # ------- S2: derived cols -------
        def dc3(c):
            return dense[:, :, c]
        nc.vector.tensor_scalar(dc3(20), dc3(2), 0.5, None, Alu.mult)
        nc.vector.tensor_scalar(dc3(21), dc3(3), 0.5, None, Alu.mult)
        nc.vector.tensor_tensor(dc3(10), dc3(0), dc3(20), Alu.subtract)
        nc.vector.tensor_tensor(dc3(11), dc3(0), dc3(20), Alu.add)
        nc.vector.tensor_tensor(dc3(12), dc3(1), dc3(21), Alu.subtract)
        nc.vector.tensor_tensor(dc3(13), dc3(1), dc3(21), Alu.add)
        nc.vector.tensor_tensor(dc3(14), dc3(2), dc3(3), Alu.mult)
        nc.vector.tensor_scalar(dc3(15), dc3(4), float(THR), None, Alu.is_gt)  # lf
        nc.vector.tensor_scalar(dc3(16), dc3(8), float(THR), None, Alu.is_gt)  # rf
        nc.vector.tensor_scalar(dc3(17), dc3(5), float(THR), None, Alu.is_gt)  # la
        nc.vector.tensor_scalar(dc3(18), dc3(9), float(THR), None, Alu.is_gt)  # ra
        awf = pd.tile([128, EW, 2], bf16)
        nc.vector.tensor_copy(awf[:, :, 0], dc3(17))
        nc.vector.tensor_copy(awf[:, :, 1], dc3(18))

        # ------- S3: transposes -> planes (0 x1,1 x2,2 y1,3 y2,4 area,5 lf,6 rf) -------
        plane = pb.tile([5, NROW], f32)
        for b in range(B_LOC):
            for half in range(2):
                tp = pp.tile([7, 512], f32, tag="tp_any")
                width = 0
                for ji, jt in enumerate(range(4 * half, 4 * half + 4)):
                    s = 8 * b + jt
                    n = 104 if jt == 7 else 128
                    nc.tensor.transpose(
                        tp[0:5, 128 * ji:128 * ji + n],
                        dense[0:n, s, 10:15], idn[0:n, 0:n])
                    width += n
                c0 = 1000 * b + 512 * half
                nc.scalar.copy(plane[0:5, c0:c0 + width], tp[0:5, 0:width])

        # ------- S5: pairwise + PE reduce -------
        nc.sync.dma_start(bounce2[:, :], plane[:, :])
        for b in range(B_LOC):
            bcs = []
            for q in range(5):
                t = psc.tile([128, 1008], f32, tag=f"bc{q}")
                nc.sync.dma_start(
                    t[:, :], bounce2[q, 1000 * b:1000 * b + 1008].partition_broadcast(128))
                bcs.append(t)
            bx1, bx2, by1, by2, bar = bcs
            pr = pp.tile([2, 1008], f32, tag="pred")
            for jt in range(8):
                s = 8 * b + jt
                n = 104 if jt == 7 else 128
                j = lambda c: dense[0:n, s, c:c + 1]
                a = psc.tile([128, 1008], f32, tag="sa")
                bq = psc.tile([128, 1008], f32, tag="sb")
                cq = psc.tile([128, 1008], f32, tag="sc")
                dq = psc.tile([128, 1008], f32, tag="sd")
                nc.vector.tensor_scalar(a[0:n, :], bx1[0:n, :], j(10), None, Alu.max)
                nc.vector.tensor_scalar(bq[0:n, :], bx2[0:n, :], j(11), None, Alu.min)
                nc.vector.tensor_tensor(bq[0:n, :], bq[0:n, :], a[0:n, :], Alu.subtract)
                nc.scalar.activation(a[0:n, :], bq[0:n, :], Act.Relu)
                nc.vector.tensor_scalar(cq[0:n, :], by1[0:n, :], j(12), None, Alu.max)
                nc.vector.tensor_scalar(dq[0:n, :], by2[0:n, :], j(13), None, Alu.min)
                nc.vector.tensor_tensor(dq[0:n, :], dq[0:n, :], cq[0:n, :], Alu.subtract)
                nc.scalar.activation(cq[0:n, :], dq[0:n, :], Act.Relu)
                nc.vector.tensor_tensor(a[0:n, :], a[0:n, :], cq[0:n, :], Alu.mult)  # inter
                nc.scalar.activation(cq[0:n, :], bar[0:n, :], Act.Identity, bias=j(14))  # S
                nc.vector.tensor_tensor(cq[0:n, :], cq[0:n, :], a[0:n, :], Alu.subtract)  # U
                nc.scalar.mul(a[0:n, :], a[0:n, :], 2.0)  # 2*inter
                r01 = pw.tile([128, 1008], bf16, tag="r01")
                nc.vector.tensor_tensor(r01[0:n, :], cq[0:n, :], a[0:n, :], Alu.is_lt)
                nc.tensor.matmul(pr[:, 0:512], awf[0:n, s, :], r01[0:n, 0:512],
                                 start=(jt == 0), stop=(jt == 7))
                nc.tensor.matmul(pr[:, 512:1008], awf[0:n, s, :], r01[0:n, 512:1008],
                                 start=(jt == 0), stop=(jt == 7), skip_group_check=True)
            mgt = ps_.tile([2, 1008], f32, tag="mgt")
            nc.scalar.activation(mgt[0:2, :], pr[0:2, :], Act.Sign)
            pt = pp.tile([128, 16], f32, tag="tp_any")
            for jt in range(8):
                n = 104 if jt == 7 else 128
                nc.tensor.transpose(pt[0:n, 2 * jt:2 * jt + 2],
                                    mgt[0:2, 128 * jt:128 * jt + n], idn[0:2, 0:2])
            nc.vector.tensor_copy(
                dense[:, 8 * b:8 * b + 8, 28:30],
                pt[:, :].rearrange("p (s c) -> p s c", s=8))

        # ------- S6: cand mask in dense cols: 28 hl 29 hr -> 30 cl 31 cr 32 candor ---
        nc.vector.tensor_scalar(dc3(33), dc3(28), -1.0, 1.0, Alu.mult, Alu.add)
        nc.vector.tensor_tensor(dc3(30), dc3(15), dc3(33), Alu.mult)
        nc.vector.tensor_scalar(dc3(33), dc3(29), -1.0, 1.0, Alu.mult, Alu.add)
        nc.vector.tensor_tensor(dc3(31), dc3(16), dc3(33), Alu.mult)
        nc.vector.tensor_tensor(dc3(32), dc3(30), dc3(31), Alu.max)

        # ------- S8: bounce write -------
        for (s, r0, n) in slot_rows:
            nc.sync.dma_start(bounce[r0:r0 + n, :], dense[0:n, s, :])

        # ------- S9: candidate indices -------
        cwl = ps_.tile([16, 504], f32, tag="cwl")
        for b in range(B_LOC):
            nc.sync.dma_start(
                cwl[:, 63 * b:63 * b + 63],
                bounce[1000 * b:1000 * b + 1008, 32:33].rearrange("(c p) o -> (o p) c", p=16))
        iop = pc.tile([16, 504], f32)
        nc.sync.dma_start(iop[:], iotap1w[:, :])
        iow = pc.tile([16, 16], f32)
        nc.sync.dma_start(iow[:], iotaw[:, :])
        iv = ps_.tile([16, 504], f32, tag="iv")
        nc.vector.tensor_tensor(iv[:, :], cwl[:, :], iop[:, :], Alu.mult)
        nc.vector.tensor_scalar(iv[:, :], iv[:, :], 1.0, None, Alu.subtract)

        sgo = ps_.tile([16, 128], f32, tag="sgo")
        nc.vector.memset(sgo[:], -1.0)
        cnts = ps_.tile([1, 8], u32, tag="cnts")
        nc.gpsimd.memset(cnts[:], 0)
        sload(library_config.sparse_gather)
        for b in range(B_LOC):
            sdep(nc.gpsimd.sparse_gather(
                sgo[:, 16 * b:16 * b + 16], iv[:, 63 * b:63 * b + 63],
                num_found=cnts[0:1, b:b + 1]))

        cntf = ps_.tile([1, 8], f32, tag="cntf")
        nc.vector.tensor_copy(cntf[:], cnts[:])
        sload(library_config.mlp)
        cnb = ps_.tile([16, 8], f32, tag="cnb")
        sdep(nc.gpsimd.partition_broadcast(cnb[:], cntf[:]))

        val = ps_.tile([16, 128], f32, tag="val")
        gfi = ps_.tile([16, 128], f32, tag="gfi")
        sci = ps_.tile([16, 128], f32, tag="sci")
        for b in range(B_LOC):
            sl = (slice(0, 16), slice(16 * b, 16 * b + 16))
            nc.vector.tensor_scalar(val[sl], iow[:, :], cnb[:, b:b + 1], None, Alu.is_lt)
            nc.vector.tensor_scalar(gfi[sl], sgo[sl], float(1000 * b - 8000), None, Alu.add)
            nc.vector.tensor_tensor(gfi[sl], gfi[sl], val[sl], Alu.mult)
            nc.vector.tensor_scalar(gfi[sl], gfi[sl], 8000.0, None, Alu.add)
            nc.vector.tensor_scalar(sci[sl], sgo[sl], 1000.0, None, Alu.subtract)
            nc.vector.tensor_tensor(sci[sl], sci[sl], val[sl], Alu.mult)
            nc.vector.tensor_scalar(sci[sl], sci[sl], float(1000 + 1001 * b), None, Alu.add)
        gfi16 = ps_.tile([16, 128], i16, tag="gfi16")
        sci16 = ps_.tile([16, 128], i16, tag="sci16")
        nc.vector.tensor_copy(gfi16[:], gfi[:])
        nc.vector.tensor_copy(sci16[:], sci[:])
        gfir = ps_.tile([128, 128], i16, tag="gfir")
        scir = ps_.tile([128, 128], i16, tag="scir")
        for g in range(8):
            nc.sync.dma_start(gfir[16 * g:16 * g + 16, :], gfi16[:, :])
            nc.sync.dma_start(scir[16 * g:16 * g + 16, :], sci16[:, :])

        # ------- S10: gathers -------
        frhs_t = []
        crow = pb.tile([128, 16, EW], f32)
        for b in range(B_LOC):
            ft = pd.tile([128, 2, 256], bf16, tag=f"frhs{b}")
            sdep(nc.gpsimd.dma_gather(
                ft[:, :, :], featbf[:, :],
                gfir[:, 16 * b:16 * b + 16], CANDP, CANDP, D, transpose=True))
            frhs_t.append(ft)
            sdep(nc.gpsimd.dma_gather(
                crow[:, 2 * b:2 * b + 2, :], bounce[:, :],
                gfir[:, 16 * b:16 * b + 16], CANDP, CANDP, EW))

        # ------- S11: MLP -------
        wsb = {}
        for nm, ap_ in [("wp1", wp1), ("wp2", wp2), ("wb1", wb1), ("wb2", wb2)]:
            t = pc.tile([128, 2, D], bf16, tag=f"w_{nm}")
            nc.sync.dma_start(t[:, 0, :], ap_[0:128, :])
            nc.sync.dma_start(t[:, 1, :], ap_[128:256, :])
            wsb[nm] = t
        whd_sb = pc.tile([128, 2, 8], bf16)
        nc.sync.dma_start(whd_sb[:, 0, :], whd[0:128, :])
        nc.sync.dma_start(whd_sb[:, 1, :], whd[128:256, :])
        # cvecT rows: per-channel cols: 0 bp1 1 bp2 2 bb1 3 bb2 4 lng 5 lnb 6 bhead 7 bs0
        cv = pc.tile([128, 2, 8], f32)
        nc.sync.dma_start(cv[:, 0, :], cvecT[0:128, :])
        nc.sync.dma_start(cv[:, 1, :], cvecT[128:256, :])
        ones_bf = pc.tile([128, 1], bf16)
        nc.vector.memset(ones_bf[:], 1.0)

        y1a = pb.tile([128, 2, 2048], bf16)
        stats_y = ps_.tile([8, 256], f32, tag="stats_y")
        stats_q = ps_.tile([8, 256], f32, tag="stats_q")
        for b in range(B_LOC):
            cols = slice(256 * b, 256 * b + 256)
            ysq = ps_.tile([128, 2, 256], bf16, tag="ysq")
            for m in range(2):
                psm = pp2.tile([128, 256], f32, tag="mlp")
                for k in range(2):
                    nc.tensor.matmul(psm[:, :], wsb["wp1"][:, k, 128 * m:128 * m + 128],
                                     frhs_t[b][:, k, :], start=(k == 0), stop=(k == 1))
                nc.scalar.activation(y1a[:, m, cols], psm[:, :], Act.Identity, bias=cv[:, m, 0:1])
                nc.scalar.activation(ysq[:, m, :], psm[:, :], Act.Square, bias=cv[:, m, 0:1])
            st0 = pp.tile([1, 256], f32, tag="stA")
            st1 = pp.tile([1, 256], f32, tag="stB")
            for m in range(2):
                nc.tensor.matmul(st0[:, :], ones_bf[:, :], y1a[:, m, cols],
                                 start=(m == 0), stop=(m == 1))
            for m in range(2):
                nc.tensor.matmul(st1[:, :], ones_bf[:, :], ysq[:, m, :],
                                 start=(m == 0), stop=(m == 1))
            ev0 = ps_.tile([1, 256], f32, tag="ev0")
            ev1 = ps_.tile([1, 256], f32, tag="ev1")
            nc.vector.tensor_copy(ev0[:, :], st0[:, :])
            nc.vector.tensor_copy(ev1[:, :], st1[:, :])
            nc.sync.dma_start(stats_y[b:b + 1, :], ev0[:, :])
            nc.sync.dma_start(stats_q[b:b + 1, :], ev1[:, :])

        s1 = ps_.tile([8, 256], f32, tag="s1")
        s1q = ps_.tile([8, 256], f32, tag="s1q")
        nc.vector.tensor_scalar(s1[:, :], stats_y[:, :], 1.0 / 256.0, None, Alu.mult)
        nc.vector.tensor_scalar(s1q[:, :], stats_q[:, :], 1.0 / 256.0, None, Alu.mult)
        var = ps_.tile([8, 256], f32, tag="var")
        nc.vector.tensor_tensor(var[:, :], s1[0:8, :], s1[0:8, :], Alu.mult)
        nc.vector.tensor_tensor(var[:, :], s1q[:, :], var[:, :], Alu.subtract)
        nc.vector.tensor_scalar(var[:, :], var[:, :], 1e-5, None, Alu.add)
        rec = ps_.tile([8, 256], f32, tag="rec")
        nc.vector.reciprocal(rec[:, :], var[:, :])
        rstd = ps_.tile([8, 256], f32, tag="rstd")
        nc.scalar.activation(rstd[:, :], rec[:, :], Act.Sqrt)
        lnB = ps_.tile([8, 256], f32, tag="lnB")
        nc.vector.tensor_tensor(lnB[:, :], s1[0:8, :], rstd[:, :], Alu.mult)
        nc.vector.tensor_scalar(lnB[:, :], lnB[:, :], -1.0, None, Alu.mult)

        bounce3 = nc.dram_tensor("bounce3", [16, D], f32).ap()
        nc.sync.dma_start(bounce3[0:8, :], rstd[:, :])
        nc.sync.dma_start(bounce3[8:16, :], lnB[:, :])
        delta_all = pb.tile([128, 16, 8], f32)
        for b in range(B_LOC):
            cols = slice(256 * b, 256 * b + 256)
            Ab = ps_.tile([128, 256], f32, tag="Ab")
            Bb = ps_.tile([128, 256], f32, tag="Bb")
            nc.sync.dma_start(Ab[:, :], bounce3[b, :].partition_broadcast(128))
            nc.sync.dma_start(Bb[:, :], bounce3[8 + b, :].partition_broadcast(128))
            h1t = ps_.tile([128, 2, 256], bf16, tag="h1t")
            for m in range(2):
                t1 = ps_.tile([128, 256], f32, tag="t1")
                nc.vector.tensor_tensor(t1[:, :], y1a[:, m, cols], Ab[:, :], Alu.mult)
                nc.vector.tensor_tensor(t1[:, :], t1[:, :], Bb[:, :], Alu.add)
                nc.vector.tensor_scalar(t1[:, :], t1[:, :], cv[:, m, 4:5], cv[:, m, 5:6],
                                        Alu.mult, Alu.add)
                nc.scalar.activation(h1t[:, m, :], t1[:, :], Act.Relu)
            aft = ps_.tile([128, 2, 256], bf16, tag="aft")
            for m in range(2):
                psm = pp2.tile([128, 256], f32, tag="mlp")
                for k in range(2):
                    nc.tensor.matmul(psm[:, :], wsb["wp2"][:, k, 128 * m:128 * m + 128],
                                     h1t[:, k, :], start=(k == 0), stop=(k == 1))
                nc.scalar.activation(aft[:, m, :], psm[:, :], Act.Identity, bias=cv[:, m, 1:2])
            m1t = ps_.tile([128, 2, 256], bf16, tag="m1t")
            for m in range(2):
                psm = pp2.tile([128, 256], f32, tag="mlp")
                for k in range(2):
                    nc.tensor.matmul(psm[:, :], wsb["wb1"][:, k, 128 * m:128 * m + 128],
                                     aft[:, k, :], start=(k == 0), stop=(k == 1))
                nc.scalar.activation(m1t[:, m, :], psm[:, :], Act.Relu, bias=cv[:, m, 2:3])
            m2t = ps_.tile([128, 2, 256], bf16, tag="m2t")
            for m in range(2):
                psm = pp2.tile([128, 256], f32, tag="mlp")
                for k in range(2):
                    nc.tensor.matmul(psm[:, :], wsb["wb2"][:, k, 128 * m:128 * m + 128],
                                     m1t[:, k, :], start=(k == 0), stop=(k == 1))
                nc.scalar.activation(m2t[:, m, :], psm[:, :], Act.Relu, bias=cv[:, m, 3:4])
            hd1 = pp.tile([4, 256], f32, tag="stA")
            hd2 = pp.tile([1, 256], f32, tag="stB")
            for k in range(2):
                nc.tensor.matmul(hd1[:, :], whd_sb[:, k, 0:4], m2t[:, k, :],
                                 start=(k == 0), stop=(k == 1))
            for k in range(2):
                nc.tensor.matmul(hd2[:, :], whd_sb[:, k, 4:5], aft[:, k, :],
                                 start=(k == 0), stop=(k == 1))
            hs1 = ps_.tile([4, 256], f32, tag="hs1")
            hs2 = ps_.tile([1, 256], f32, tag="hs2")
            nc.scalar.activation(hs1[:, :], hd1[:, :], Act.Identity, bias=cv[0:4, 0, 6:7])
            nc.scalar.activation(hs2[:, :], hd2[:, :], Act.Identity, bias=cv[0:1, 0, 7:8])
            tph = pp.tile([128, 2, 5], f32, tag="tp_any")
            for half in range(2):
                nc.tensor.transpose(tph[:, half, 0:4], hs1[0:4, 128 * half:128 * half + 128], idn[0:4, 0:4])
                nc.tensor.transpose(tph[:, half, 4:5], hs2[0:1, 128 * half:128 * half + 128], idn[0:1, 0:1])
            nc.vector.tensor_copy(delta_all[:, 2 * b:2 * b + 2, 0:5], tph[:, :, :])

        # ------- S13: value path on candidate rows -------
        cr3 = lambda c: crow[:, :, c]
        nc.vector.tensor_scalar(cr3(33), cr3(3), 0.5, None, Alu.mult)
        nc.vector.tensor_tensor(cr3(32), cr3(1), cr3(33), Alu.add)
        nc.vector.tensor_scalar(cr3(34), cr3(2), 0.8, None, Alu.mult)
        nc.vector.tensor_scalar(cr3(35), cr3(3), 0.8, None, Alu.mult)
        nc.vector.tensor_scalar(cr3(37), cr3(2), 0.1, None, Alu.mult)
        nc.vector.tensor_tensor(cr3(36), cr3(0), cr3(37), Alu.add)
        srcs = [0, 32, 34, 35, 36, 32, 34, 35]
        for i, c in enumerate(srcs):
            nc.vector.tensor_scalar(cr3(40 + i), cr3(c), 1e-5, 1.0, Alu.max, Alu.min)
            nc.vector.tensor_scalar(cr3(48 + i), cr3(c), -1.0, 1.0, Alu.mult, Alu.add)
            nc.vector.tensor_scalar(cr3(48 + i), cr3(48 + i), 1e-5, 1.0, Alu.max, Alu.min)
        lg = crow[:, :, 40:56]
        nc.scalar.activation(lg, lg, Act.Ln)
        nc.vector.tensor_tensor(
            crow[:, :, 56:64], crow[:, :, 40:48],
            crow[:, :, 48:56], Alu.subtract)

        u = crow[:, :, 10:14]
        nc.vector.tensor_tensor(u, crow[:, :, 56:60], delta_all[:, :, 0:4], Alu.add)
        u2 = crow[:, :, 15:19]
        nc.vector.tensor_tensor(u2, crow[:, :, 60:64], delta_all[:, :, 0:4], Alu.add)
        nc.scalar.activation(u, u, Act.Sigmoid)
        nc.scalar.activation(u2, u2, Act.Sigmoid)
        nc.vector.tensor_copy(crow[:, :, 14], delta_all[:, :, 4])
        nc.vector.tensor_copy(crow[:, :, 19], delta_all[:, :, 4])
        for c in range(5):
            nc.vector.tensor_tensor(crow[:, :, 10 + c], crow[:, :, 10 + c], cr3(30), Alu.mult)
            nc.vector.tensor_tensor(crow[:, :, 15 + c], crow[:, :, 15 + c], cr3(31), Alu.mult)

        # ------- S14: scatter -------
        sdep(nc.gpsimd.dma_scatter_add(
            canvas[:, :], crow[:, :, :], scir[:, :], 2048, 2048, EW))

    nc.compile()
    return nc


def _prep_consts():
    iotap1w = np.zeros((16, 504), np.float32)
    for b in range(8):
        for k in range(1000):
            iotap1w[k % 16, 63 * b + k // 16] = k + 1
    iotaw = np.zeros((16, 16), np.float32)
    for k in range(256):
        iotaw[k % 16, k // 16] = k
    return iotap1w, iotaw


def make_in_maps(inputs):
    logits = np.ascontiguousarray(inputs["logits"], np.float32)
    boxes = np.ascontiguousarray(inputs["boxes"], np.float32)
    features = np.ascontiguousarray(inputs["features"], np.float32)
    n_cores = 8
    bpc = logits.shape[0] // n_cores
    iotap1w, iotaw = _prep_consts()
    bf = ml_dtypes.bfloat16
    wp1 = inputs["W_p1"].astype(bf); wp2 = inputs["W_p2"].astype(bf)
    wb1 = inputs["W_b1"].astype(bf); wb2 = inputs["W_b2"].astype(bf)
    whd = np.zeros((D, 8), np.float32)
    whd[:, 0:4] = inputs["W_b3"]; whd[:, 4:5] = inputs["W_s"]
    whd = whd.astype(bf)
    cvecT = np.zeros((D, 8), np.float32)
    cvecT[:, 0] = inputs["b_p1"]; cvecT[:, 1] = inputs["b_p2"]
    cvecT[:, 2] = inputs["b_b1"]; cvecT[:, 3] = inputs["b_b2"]
    cvecT[:, 4] = inputs["ln_g"]; cvecT[:, 5] = inputs["ln_b"]
    cvecT[0:4, 6] = inputs["b_b3"]; cvecT[0, 7] = inputs["b_s"][0]
    in_maps = []
    for c in range(n_cores):
        sl = slice(c * bpc, (c + 1) * bpc)
        bl = np.zeros((NROW, EW), np.float32)
        bl[:8000, 0:4] = boxes[sl].reshape(-1, 4)
        bl[:8000, 4:10] = logits[sl].reshape(-1, 6)
        fb = np.zeros((8001, D), bf)
        fb[:8000] = features[sl].reshape(-1, D).astype(bf)
        in_maps.append(dict(
            boxlog=bl, featbf=fb, wp1=wp1, wp2=wp2, wb1=wb1, wb2=wb2,
            whd=whd, cvecT=cvecT, iotap1w=iotap1w, iotaw=iotaw,
            identd=np.eye(128, dtype=np.float32)))
    return in_maps, bpc


def unpack(results, Bf, bpc):
    out = np.zeros((Bf, Q, 2, 5), np.float32)
    mask = np.zeros((Bf, Q, 2), bool)
    for c in range(len(results)):
        r = results[c]
        cvs = r["canvas"]; mk = r["bounce"]
        for b in range(bpc):
            rows = cvs[1001 * b:1001 * b + 1000, 10:20]
            out[c * bpc + b, :, 0, :] = rows[:, 0:5]
            out[c * bpc + b, :, 1, :] = rows[:, 5:10]
            mask[c * bpc + b, :, 0] = mk[1000 * b:1000 * b + 1000, 30] > 0.5
            mask[c * bpc + b, :, 1] = mk[1000 * b:1000 * b + 1000, 31] > 0.5
    return out, mask


def kernel(**inputs):
    if "nc" not in _CACHED:
        _CACHED["nc"] = build()
    nc = _CACHED["nc"]
    in_maps, bpc = make_in_maps(inputs)
    res = run_bass_kernel_spmd(nc, in_maps, core_ids=list(range(8)))
    return unpack(res.results, inputs["logits"].shape[0], bpc)


# revision 19
# speedup vs baseline: 1.1638x; 1.1638x over previous
"""Trainium2 Bass kernel for nn_AnatomicalRelationshipModule.

Data-parallel over 8 NeuronCores (8 images each). Per core:
  - dense per-box quantities + PE transposes to row-major planes
  - pairwise IoU>0.5 test (exact fp32 replica of reference math) with
    PE matmul OR-reduction against arytenoid flags
  - candidate mask, stream-compaction of candidate rows (sparse_gather)
  - MLP refinement only on gathered candidate rows (dma_gather, bf16 PE)
  - output assembled by dma_scatter_add onto a zeroed canvas
"""
import sys
import numpy as np

sys.path.insert(0, '/opt/trn_rl_repo')

import concourse.bass as bass
import concourse.bacc as bacc
import concourse.mybir as mybir
import concourse.tile as tile
from concourse import library_config
from concourse.bass import _add_dep_helper
from concourse.bass_utils import run_bass_kernel_spmd
from contextlib import ExitStack
import ml_dtypes

f32 = mybir.dt.float32
bf16 = mybir.dt.bfloat16
i16 = mybir.dt.int16
u32 = mybir.dt.uint32
Alu = mybir.AluOpType
Act = mybir.ActivationFunctionType

B_LOC = 8
Q = 1000
D = 256
THR = np.float32(0.84729785)   # fp32 boundary of sigmoid(x) > 0.7
CANDP = 256
NROW = 8064
EW = 64

_CACHED = {}


def build():
    nc = bacc.Bacc("TRN2", target_bir_lowering=False, debug=False)
    boxlog = nc.dram_tensor("boxlog", [NROW, EW], f32, kind="ExternalInput").ap()
    featbf = nc.dram_tensor("featbf", [8001, D], bf16, kind="ExternalInput").ap()
    wp1 = nc.dram_tensor("wp1", [D, D], bf16, kind="ExternalInput").ap()
    wp2 = nc.dram_tensor("wp2", [D, D], bf16, kind="ExternalInput").ap()
    wb1 = nc.dram_tensor("wb1", [D, D], bf16, kind="ExternalInput").ap()
    wb2 = nc.dram_tensor("wb2", [D, D], bf16, kind="ExternalInput").ap()
    whd = nc.dram_tensor("whd", [D, 8], bf16, kind="ExternalInput").ap()
    cvecT = nc.dram_tensor("cvecT", [D, 8], f32, kind="ExternalInput").ap()
    iotap1w = nc.dram_tensor("iotap1w", [16, 504], f32, kind="ExternalInput").ap()
    iotaw = nc.dram_tensor("iotaw", [16, 16], f32, kind="ExternalInput").ap()
    iotaw2 = nc.dram_tensor("iotaw2", [16, 32], f32, kind="ExternalInput").ap()
    identd = nc.dram_tensor("identd", [128, 128], f32, kind="ExternalInput").ap()

    canvas = nc.dram_tensor("canvas", [8192, EW], f32, kind="ExternalOutput").ap()
    bounce = nc.dram_tensor("bounce", [NROW, EW], f32, kind="ExternalOutput").ap()
    bounce2 = nc.dram_tensor("bounce2", [6, NROW], f32).ap()

    seq = [None, []]

    def sdep(binst):
        if seq[0] is not None:
            _add_dep_helper(binst.ins, seq[0].ins, True, "lib dep")
        seq[1].append(binst)
        return binst

    def sload(lib):
        b = nc.gpsimd.load_library(lib)
        for p in seq[1]:
            _add_dep_helper(b.ins, p.ins, True, "lib order")
        seq[0] = b
        seq[1] = []
        return b

    with tile.TileContext(nc) as tc, ExitStack() as ctx:
        pc = ctx.enter_context(tc.tile_pool(name="const", bufs=1))
        pd = ctx.enter_context(tc.tile_pool(name="dense", bufs=1))
        pw = ctx.enter_context(tc.tile_pool(name="work", bufs=2))
        psc = ctx.enter_context(tc.tile_pool(name="scratch", bufs=1))
        ps_ = ctx.enter_context(tc.tile_pool(name="small", bufs=2))
        pp = ctx.enter_context(tc.tile_pool(name="psum", bufs=1, space="PSUM"))
        pp2 = ctx.enter_context(tc.tile_pool(name="psum2", bufs=2, space="PSUM"))
        pb = ctx.enter_context(tc.tile_pool(name="big", bufs=1))

        # ------- S0: zero canvas -------
        idn = pc.tile([128, 128], f32)
        nc.sync.dma_start(idn[:], identd[:, :])
        zsb = pc.tile([128, 512], f32)
        nc.vector.memset(zsb[:], 0.0)
        for c in range(8):
            nc.sync.dma_start(canvas[1024 * c:1024 * (c + 1), :], zsb[:])

        # ------- S1: dense load -------
        # cols: 0 cx 1 cy 2 w 3 h 4..9 logits | 10 x1 11 x2 12 y1 13 y2 14 area
        #       15 lf 16 rf 17 la 18 ra 20 halfw 21 halfh 30 cl 31 cr
        dense = pd.tile([128, EW, EW], f32)
        nc.vector.memset(dense[:], 0.0)
        slot_rows = []
        for b in range(B_LOC):
            for jt in range(8):
                s = 8 * b + jt
                r0 = 1000 * b + 128 * jt
                n = 104 if jt == 7 else 128
                slot_rows.append((s, r0, n))
                nc.sync.dma_start(dense[0:n, s, :], boxlog[r0:r0 + n, :])

        # ------- S2: derived cols -------
        def dc3(c):
            return dense[:, :, c]
        nc.vector.tensor_scalar(dc3(20), dc3(2), 0.5, None, Alu.mult)
        nc.vector.tensor_scalar(dc3(21), dc3(3), 0.5, None, Alu.mult)
        nc.vector.tensor_tensor(dc3(10), dc3(0), dc3(20), Alu.subtract)
        nc.vector.tensor_tensor(dc3(11), dc3(0), dc3(20), Alu.add)
        nc.vector.tensor_tensor(dc3(12), dc3(1), dc3(21), Alu.subtract)
        nc.vector.tensor_tensor(dc3(13), dc3(1), dc3(21), Alu.add)
        nc.vector.tensor_tensor(dc3(14), dc3(2), dc3(3), Alu.mult)
        nc.vector.tensor_scalar(dc3(16), dc3(4), float(THR), None, Alu.is_gt)  # lf
        nc.vector.tensor_scalar(dc3(17), dc3(8), float(THR), None, Alu.is_gt)  # rf
        nc.vector.tensor_scalar(dc3(18), dc3(5), float(THR), None, Alu.is_gt)  # la
        nc.vector.tensor_scalar(dc3(19), dc3(9), float(THR), None, Alu.is_gt)  # ra
        nc.vector.tensor_tensor(dc3(15), dc3(18), dc3(19), Alu.max)  # aryor

        # ------- S3: transposes -> planes (0 x1,1 x2,2 y1,3 y2,4 area,5 lf,6 rf) -------
        plane = pb.tile([6, NROW], f32)
        nc.vector.memset(plane[0:6, 8000:NROW], 0.0)
        for b in range(B_LOC):
            for half in range(2):
                tp = pp.tile([7, 512], f32, tag="tp_any")
                width = 0
                for ji, jt in enumerate(range(4 * half, 4 * half + 4)):
                    s = 8 * b + jt
                    n = 104 if jt == 7 else 128
                    nc.tensor.transpose(
                        tp[0:6, 128 * ji:128 * ji + n],
                        dense[0:n, s, 10:16], idn[0:n, 0:n])
                    width += n
                c0 = 1000 * b + 512 * half
                nc.scalar.copy(plane[0:6, c0:c0 + width], tp[0:6, 0:width])

        # ------- S4b: ary compaction -------
        nc.sync.dma_start(bounce2[:, :], plane[:, :])
        iopA = pc.tile([16, 504], f32, tag="iopA")
        nc.sync.dma_start(iopA[:], iotap1w[:, :])
        iowA = pc.tile([16, 32], f32, tag="iowA")
        nc.sync.dma_start(iowA[:], iotaw2[:, :])
        awj = ps_.tile([16, 504], f32, tag="awj")
        for b in range(B_LOC):
            nc.sync.dma_start(
                awj[:, 63 * b:63 * b + 63],
                bounce2[5, 1000 * b:1000 * b + 1008].rearrange("(c p) -> p c", p=16))
        ivA = ps_.tile([16, 504], f32, tag="ivA")
        nc.vector.tensor_tensor(ivA[:, :], awj[:, :], iopA[:, :], Alu.mult)
        nc.vector.tensor_scalar(ivA[:, :], ivA[:, :], 1.0, None, Alu.subtract)
        sgoA = ps_.tile([16, 256], f32, tag="sgoA")
        nc.vector.memset(sgoA[:], -1.0)
        cntsA = ps_.tile([1, 8], u32, tag="cntsA")
        nc.gpsimd.memset(cntsA[:], 0)
        sload(library_config.sparse_gather)
        for b in range(B_LOC):
            sdep(nc.gpsimd.sparse_gather(
                sgoA[:, 32 * b:32 * b + 32], ivA[:, 63 * b:63 * b + 63],
                num_found=cntsA[0:1, b:b + 1]))
        cntfA = ps_.tile([1, 8], f32, tag="cntfA")
        nc.vector.tensor_copy(cntfA[:], cntsA[:])
        sload(library_config.mlp)
        cnbA = ps_.tile([16, 8], f32, tag="cnbA")
        sdep(nc.gpsimd.partition_broadcast(cnbA[:], cntfA[:]))
        valA = ps_.tile([16, 256], f32, tag="valA")
        gaj = ps_.tile([16, 256], f32, tag="gaj")
        for b in range(B_LOC):
            sl = (slice(0, 16), slice(32 * b, 32 * b + 32))
            nc.vector.tensor_scalar(valA[sl], iowA[:, :], cnbA[:, b:b + 1], None, Alu.is_lt)
            nc.vector.tensor_scalar(gaj[sl], sgoA[sl], float(1000 * b - 8000), None, Alu.add)
            nc.vector.tensor_tensor(gaj[sl], gaj[sl], valA[sl], Alu.mult)
            nc.vector.tensor_scalar(gaj[sl], gaj[sl], 8000.0, None, Alu.add)
        gaj16 = ps_.tile([16, 256], i16, tag="gaj16")
        nc.vector.tensor_copy(gaj16[:], gaj[:])
        gajr = ps_.tile([128, 256], i16, tag="gajr")
        for g in range(8):
            nc.sync.dma_start(gajr[16 * g:16 * g + 16, :], gaj16[:, :])
        jrow = pd.tile([128, 32, EW], f32)
        nc.vector.memset(jrow[:], 0.0)
        for b in range(B_LOC):
            sdep(nc.gpsimd.dma_gather(
                jrow[:, 4 * b:4 * b + 4, :], boxlog[:, :],
                gajr[:, 32 * b:32 * b + 32], 512, 512, EW))
        # derived cols on gathered j rows
        def jc3(c):
            return jrow[:, :, c]
        nc.vector.tensor_scalar(jc3(20), jc3(2), 0.5, None, Alu.mult)
        nc.vector.tensor_scalar(jc3(21), jc3(3), 0.5, None, Alu.mult)
        nc.vector.tensor_tensor(jc3(10), jc3(0), jc3(20), Alu.subtract)
        nc.vector.tensor_tensor(jc3(11), jc3(0), jc3(20), Alu.add)
        nc.vector.tensor_tensor(jc3(12), jc3(1), jc3(21), Alu.subtract)
        nc.vector.tensor_tensor(jc3(13), jc3(1), jc3(21), Alu.add)
        nc.vector.tensor_tensor(jc3(14), jc3(2), jc3(3), Alu.mult)
        nc.vector.tensor_scalar(jc3(18), jc3(5), float(THR), None, Alu.is_gt)
        nc.vector.tensor_scalar(jc3(19), jc3(9), float(THR), None, Alu.is_gt)
        awf = pd.tile([128, 32, 2], bf16)
        nc.vector.tensor_copy(awf[:, :, 0], jc3(18))
        nc.vector.tensor_copy(awf[:, :, 1], jc3(19))

        # ------- S5: pairwise + PE reduce -------
        for b in range(B_LOC):
            bcs = []
            for q in range(5):
                t = psc.tile([128, 1008], f32, tag=f"bc{q}")
                nc.sync.dma_start(
                    t[:, :], bounce2[q, 1000 * b:1000 * b + 1008].partition_broadcast(128))
                bcs.append(t)
            bx1, bx2, by1, by2, bar = bcs
            pr = pp.tile([2, 1008], f32, tag="pred")
            for jt in range(4):
                s = 4 * b + jt
                n = 128
                j = lambda c: jrow[0:n, s, c:c + 1]
                a = psc.tile([128, 1008], f32, tag="sa")
                bq = psc.tile([128, 1008], f32, tag="sb")
                cq = psc.tile([128, 1008], f32, tag="sc")
                dq = psc.tile([128, 1008], f32, tag="sd")
                nc.vector.tensor_scalar(a[0:n, :], bx1[0:n, :], j(10), None, Alu.max)
                nc.vector.tensor_scalar(bq[0:n, :], bx2[0:n, :], j(11), None, Alu.min)
                nc.vector.tensor_tensor(bq[0:n, :], bq[0:n, :], a[0:n, :], Alu.subtract)
                nc.scalar.activation(a[0:n, :], bq[0:n, :], Act.Relu)
                nc.vector.tensor_scalar(cq[0:n, :], by1[0:n, :], j(12), None, Alu.max)
                nc.vector.tensor_scalar(dq[0:n, :], by2[0:n, :], j(13), None, Alu.min)
                nc.vector.tensor_tensor(dq[0:n, :], dq[0:n, :], cq[0:n, :], Alu.subtract)
                nc.scalar.activation(cq[0:n, :], dq[0:n, :], Act.Relu)
                nc.vector.tensor_tensor(a[0:n, :], a[0:n, :], cq[0:n, :], Alu.mult)  # inter
                nc.scalar.activation(cq[0:n, :], bar[0:n, :], Act.Identity, bias=j(14))  # S
                nc.vector.tensor_tensor(cq[0:n, :], cq[0:n, :], a[0:n, :], Alu.subtract)  # U
                nc.scalar.mul(a[0:n, :], a[0:n, :], 2.0)  # 2*inter
                r01 = pw.tile([128, 1008], bf16, tag="r01")
                nc.vector.tensor_tensor(r01[0:n, :], cq[0:n, :], a[0:n, :], Alu.is_lt)
                nc.tensor.matmul(pr[:, 0:512], awf[0:n, s, :], r01[0:n, 0:512],
                                 start=(jt == 0), stop=(jt == 3))
                nc.tensor.matmul(pr[:, 512:1008], awf[0:n, s, :], r01[0:n, 512:1008],
                                 start=(jt == 0), stop=(jt == 3), skip_group_check=True)
            mgt = ps_.tile([2, 1008], f32, tag="mgt")
            nc.scalar.activation(mgt[0:2, :], pr[0:2, :], Act.Sign)
            pt = pp.tile([128, 16], f32, tag="tp_any")
            nc.vector.memset(pt[:], 0.0)
            for jt in range(8):
                n = 104 if jt == 7 else 128
                nc.tensor.transpose(pt[0:n, 2 * jt:2 * jt + 2],
                                    mgt[0:2, 128 * jt:128 * jt + n], idn[0:2, 0:2])
            nc.vector.tensor_copy(
                dense[:, 8 * b:8 * b + 8, 28:30],
                pt[:, :].rearrange("p (s c) -> p s c", s=8))

        # ------- S6: cand mask in dense cols: 28 hl 29 hr -> 30 cl 31 cr 32 candor ---
        nc.vector.tensor_scalar(dc3(33), dc3(28), -1.0, 1.0, Alu.mult, Alu.add)
        nc.vector.tensor_tensor(dc3(30), dc3(16), dc3(33), Alu.mult)
        nc.vector.tensor_scalar(dc3(33), dc3(29), -1.0, 1.0, Alu.mult, Alu.add)
        nc.vector.tensor_tensor(dc3(31), dc3(17), dc3(33), Alu.mult)
        nc.vector.tensor_tensor(dc3(32), dc3(30), dc3(31), Alu.max)

        # ------- S8: bounce write -------
        nc.sync.dma_start(bounce[8000:NROW, :], zsb[0:64, 0:64])
        for (s, r0, n) in slot_rows:
            nc.sync.dma_start(bounce[r0:r0 + n, :], dense[0:n, s, :])

        # ------- S9: candidate indices -------
        cwl = ps_.tile([16, 504], f32, tag="cwl")
        for b in range(B_LOC):
            nc.sync.dma_start(
                cwl[:, 63 * b:63 * b + 63],
                bounce[1000 * b:1000 * b + 1008, 32:33].rearrange("(c p) o -> (o p) c", p=16))
        iop = pc.tile([16, 504], f32)
        nc.sync.dma_start(iop[:], iotap1w[:, :])
        iow = pc.tile([16, 16], f32)
        nc.sync.dma_start(iow[:], iotaw[:, :])
        iv = ps_.tile([16, 504], f32, tag="iv")
        nc.vector.tensor_tensor(iv[:, :], cwl[:, :], iop[:, :], Alu.mult)
        nc.vector.tensor_scalar(iv[:, :], iv[:, :], 1.0, None, Alu.subtract)

        sgo = ps_.tile([16, 128], f32, tag="sgo")
        nc.vector.memset(sgo[:], -1.0)
        cnts = ps_.tile([1, 8], u32, tag="cnts")
        nc.gpsimd.memset(cnts[:], 0)
        sload(library_config.sparse_gather)
        for b in range(B_LOC):
            sdep(nc.gpsimd.sparse_gather(
                sgo[:, 16 * b:16 * b + 16], iv[:, 63 * b:63 * b + 63],
                num_found=cnts[0:1, b:b + 1]))

        cntf = ps_.tile([1, 8], f32, tag="cntf")
        nc.vector.tensor_copy(cntf[:], cnts[:])
        sload(library_config.mlp)
        cnb = ps_.tile([16, 8], f32, tag="cnb")
        sdep(nc.gpsimd.partition_broadcast(cnb[:], cntf[:]))

        val = ps_.tile([16, 128], f32, tag="val")
        gfi = ps_.tile([16, 128], f32, tag="gfi")
        sci = ps_.tile([16, 128], f32, tag="sci")
        for b in range(B_LOC):
            sl = (slice(0, 16), slice(16 * b, 16 * b + 16))
            nc.vector.tensor_scalar(val[sl], iow[:, :], cnb[:, b:b + 1], None, Alu.is_lt)
            nc.vector.tensor_scalar(gfi[sl], sgo[sl], float(1000 * b - 8000), None, Alu.add)
            nc.vector.tensor_tensor(gfi[sl], gfi[sl], val[sl], Alu.mult)
            nc.vector.tensor_scalar(gfi[sl], gfi[sl], 8000.0, None, Alu.add)
            nc.vector.tensor_scalar(sci[sl], sgo[sl], 1000.0, None, Alu.subtract)
            nc.vector.tensor_tensor(sci[sl], sci[sl], val[sl], Alu.mult)
            nc.vector.tensor_scalar(sci[sl], sci[sl], float(1000 + 1001 * b), None, Alu.add)
        gfi16 = ps_.tile([16, 128], i16, tag="gfi16")
        sci16 = ps_.tile([16, 128], i16, tag="sci16")
        nc.vector.tensor_copy(gfi16[:], gfi[:])
        nc.vector.tensor_copy(sci16[:], sci[:])
        gfir = ps_.tile([128, 128], i16, tag="gfir")
        scir = ps_.tile([128, 128], i16, tag="scir")
        for g in range(8):
            nc.sync.dma_start(gfir[16 * g:16 * g + 16, :], gfi16[:, :])
            nc.sync.dma_start(scir[16 * g:16 * g + 16, :], sci16[:, :])

        # ------- S10: gathers -------
        frhs_t = []
        crow = pb.tile([128, 16, EW], f32)
        for b in range(B_LOC):
            ft = pd.tile([128, 2, 256], bf16, tag=f"frhs{b}")
            sdep(nc.gpsimd.dma_gather(
                ft[:, :, :], featbf[:, :],
                gfir[:, 16 * b:16 * b + 16], CANDP, CANDP, D, transpose=True))
            frhs_t.append(ft)
            sdep(nc.gpsimd.dma_gather(
                crow[:, 2 * b:2 * b + 2, :], bounce[:, :],
                gfir[:, 16 * b:16 * b + 16], CANDP, CANDP, EW))

        # ------- S11: MLP -------
        wsb = {}
        for nm, ap_ in [("wp1", wp1), ("wp2", wp2), ("wb1", wb1), ("wb2", wb2)]:
            t = pc.tile([128, 2, D], bf16, tag=f"w_{nm}")
            nc.sync.dma_start(t[:, 0, :], ap_[0:128, :])
            nc.sync.dma_start(t[:, 1, :], ap_[128:256, :])
            wsb[nm] = t
        whd_sb = pc.tile([128, 2, 8], bf16)
        nc.sync.dma_start(whd_sb[:, 0, :], whd[0:128, :])
        nc.sync.dma_start(whd_sb[:, 1, :], whd[128:256, :])
        # cvecT rows: per-channel cols: 0 bp1 1 bp2 2 bb1 3 bb2 4 lng 5 lnb 6 bhead 7 bs0
        cv = pc.tile([128, 2, 8], f32)
        nc.sync.dma_start(cv[:, 0, :], cvecT[0:128, :])
        nc.sync.dma_start(cv[:, 1, :], cvecT[128:256, :])
        ones_bf = pc.tile([128, 1], bf16)
        nc.vector.memset(ones_bf[:], 1.0)

        y1a = pb.tile([128, 2, 2048], bf16)
        stats_y = ps_.tile([8, 256], f32, tag="stats_y")
        stats_q = ps_.tile([8, 256], f32, tag="stats_q")
        for b in range(B_LOC):
            cols = slice(256 * b, 256 * b + 256)
            ysq = ps_.tile([128, 2, 256], bf16, tag="ysq")
            for m in range(2):
                psm = pp2.tile([128, 256], f32, tag="mlp")
                for k in range(2):
                    nc.tensor.matmul(psm[:, :], wsb["wp1"][:, k, 128 * m:128 * m + 128],
                                     frhs_t[b][:, k, :], start=(k == 0), stop=(k == 1))
                nc.scalar.activation(y1a[:, m, cols], psm[:, :], Act.Identity, bias=cv[:, m, 0:1])
                nc.scalar.activation(ysq[:, m, :], psm[:, :], Act.Square, bias=cv[:, m, 0:1])
            st0 = pp.tile([1, 256], f32, tag="stA")
            st1 = pp.tile([1, 256], f32, tag="stB")
            for m in range(2):
                nc.tensor.matmul(st0[:, :], ones_bf[:, :], y1a[:, m, cols],
                                 start=(m == 0), stop=(m == 1))
            for m in range(2):
                nc.tensor.matmul(st1[:, :], ones_bf[:, :], ysq[:, m, :],
                                 start=(m == 0), stop=(m == 1))
            ev0 = ps_.tile([1, 256], f32, tag="ev0")
            ev1 = ps_.tile([1, 256], f32, tag="ev1")
            nc.vector.tensor_copy(ev0[:, :], st0[:, :])
            nc.vector.tensor_copy(ev1[:, :], st1[:, :])
            nc.sync.dma_start(stats_y[b:b + 1, :], ev0[:, :])
            nc.sync.dma_start(stats_q[b:b + 1, :], ev1[:, :])

        s1 = ps_.tile([8, 256], f32, tag="s1")
        s1q = ps_.tile([8, 256], f32, tag="s1q")
        nc.vector.tensor_scalar(s1[:, :], stats_y[:, :], 1.0 / 256.0, None, Alu.mult)
        nc.vector.tensor_scalar(s1q[:, :], stats_q[:, :], 1.0 / 256.0, None, Alu.mult)
        var = ps_.tile([8, 256], f32, tag="var")
        nc.vector.tensor_tensor(var[:, :], s1[0:8, :], s1[0:8, :], Alu.mult)
        nc.vector.tensor_tensor(var[:, :], s1q[:, :], var[:, :], Alu.subtract)
        nc.vector.tensor_scalar(var[:, :], var[:, :], 1e-5, None, Alu.add)
        rec = ps_.tile([8, 256], f32, tag="rec")
        nc.vector.reciprocal(rec[:, :], var[:, :])
        rstd = ps_.tile([8, 256], f32, tag="rstd")
        nc.scalar.activation(rstd[:, :], rec[:, :], Act.Sqrt)
        lnB = ps_.tile([8, 256], f32, tag="lnB")
        nc.vector.tensor_tensor(lnB[:, :], s1[0:8, :], rstd[:, :], Alu.mult)
        nc.vector.tensor_scalar(lnB[:, :], lnB[:, :], -1.0, None, Alu.mult)

        bounce3 = nc.dram_tensor("bounce3", [16, D], f32).ap()
        nc.sync.dma_start(bounce3[0:8, :], rstd[:, :])
        nc.sync.dma_start(bounce3[8:16, :], lnB[:, :])
        delta_all = pb.tile([128, 16, 8], f32)
        for b in range(B_LOC):
            cols = slice(256 * b, 256 * b + 256)
            Ab = ps_.tile([128, 256], f32, tag="Ab")
            Bb = ps_.tile([128, 256], f32, tag="Bb")
            nc.sync.dma_start(Ab[:, :], bounce3[b, :].partition_broadcast(128))
            nc.sync.dma_start(Bb[:, :], bounce3[8 + b, :].partition_broadcast(128))
            h1t = ps_.tile([128, 2, 256], bf16, tag="h1t")
            for m in range(2):
                t1 = ps_.tile([128, 256], f32, tag="t1")
                nc.vector.tensor_tensor(t1[:, :], y1a[:, m, cols], Ab[:, :], Alu.mult)
                nc.vector.tensor_tensor(t1[:, :], t1[:, :], Bb[:, :], Alu.add)
                nc.vector.tensor_scalar(t1[:, :], t1[:, :], cv[:, m, 4:5], cv[:, m, 5:6],
                                        Alu.mult, Alu.add)
                nc.scalar.activation(h1t[:, m, :], t1[:, :], Act.Relu)
            aft = ps_.tile([128, 2, 256], bf16, tag="aft")
            for m in range(2):
                psm = pp2.tile([128, 256], f32, tag="mlp")
                for k in range(2):
                    nc.tensor.matmul(psm[:, :], wsb["wp2"][:, k, 128 * m:128 * m + 128],
                                     h1t[:, k, :], start=(k == 0), stop=(k == 1))
                nc.scalar.activation(aft[:, m, :], psm[:, :], Act.Identity, bias=cv[:, m, 1:2])
            m1t = ps_.tile([128, 2, 256], bf16, tag="m1t")
            for m in range(2):
                psm = pp2.tile([128, 256], f32, tag="mlp")
                for k in range(2):
                    nc.tensor.matmul(psm[:, :], wsb["wb1"][:, k, 128 * m:128 * m + 128],
                                     aft[:, k, :], start=(k == 0), stop=(k == 1))
                nc.scalar.activation(m1t[:, m, :], psm[:, :], Act.Relu, bias=cv[:, m, 2:3])
            m2t = ps_.tile([128, 2, 256], bf16, tag="m2t")
            for m in range(2):
                psm = pp2.tile([128, 256], f32, tag="mlp")
                for k in range(2):
                    nc.tensor.matmul(psm[:, :], wsb["wb2"][:, k, 128 * m:128 * m + 128],
                                     m1t[:, k, :], start=(k == 0), stop=(k == 1))
                nc.scalar.activation(m2t[:, m, :], psm[:, :], Act.Relu, bias=cv[:, m, 3:4])
            hd1 = pp.tile([4, 256], f32, tag="stA")
            hd2 = pp.tile([1, 256], f32, tag="stB")
            for k in range(2):
                nc.tensor.matmul(hd1[:, :], whd_sb[:, k, 0:4], m2t[:, k, :],
                                 start=(k == 0), stop=(k == 1))
            for k in range(2):
                nc.tensor.matmul(hd2[:, :], whd_sb[:, k, 4:5], aft[:, k, :],
                                 start=(k == 0), stop=(k == 1))
            hs1 = ps_.tile([4, 256], f32, tag="hs1")
            hs2 = ps_.tile([1, 256], f32, tag="hs2")
            nc.scalar.activation(hs1[:, :], hd1[:, :], Act.Identity, bias=cv[0:4, 0, 6:7])
            nc.scalar.activation(hs2[:, :], hd2[:, :], Act.Identity, bias=cv[0:1, 0, 7:8])
            tph = pp.tile([128, 2, 5], f32, tag="tp_any")
            for half in range(2):
                nc.tensor.transpose(tph[:, half, 0:4], hs1[0:4, 128 * half:128 * half + 128], idn[0:4, 0:4])
                nc.tensor.transpose(tph[:, half, 4:5], hs2[0:1, 128 * half:128 * half + 128], idn[0:1, 0:1])
            nc.vector.tensor_copy(delta_all[:, 2 * b:2 * b + 2, 0:5], tph[:, :, :])

        # ------- S13: value path on candidate rows -------
        cr3 = lambda c: crow[:, :, c]
        nc.vector.tensor_scalar(cr3(33), cr3(3), 0.5, None, Alu.mult)
        nc.vector.tensor_tensor(cr3(32), cr3(1), cr3(33), Alu.add)
        nc.vector.tensor_scalar(cr3(34), cr3(2), 0.8, None, Alu.mult)
        nc.vector.tensor_scalar(cr3(35), cr3(3), 0.8, None, Alu.mult)
        nc.vector.tensor_scalar(cr3(37), cr3(2), 0.1, None, Alu.mult)
        nc.vector.tensor_tensor(cr3(36), cr3(0), cr3(37), Alu.add)
        srcs = [0, 32, 34, 35, 36, 32, 34, 35]
        for i, c in enumerate(srcs):
            nc.vector.tensor_scalar(cr3(40 + i), cr3(c), 1e-5, 1.0, Alu.max, Alu.min)
            nc.vector.tensor_scalar(cr3(48 + i), cr3(c), -1.0, 1.0, Alu.mult, Alu.add)
            nc.vector.tensor_scalar(cr3(48 + i), cr3(48 + i), 1e-5, 1.0, Alu.max, Alu.min)
        lg = crow[:, :, 40:56]
        nc.scalar.activation(lg, lg, Act.Ln)
        nc.vector.tensor_tensor(
            crow[:, :, 56:64], crow[:, :, 40:48],
            crow[:, :, 48:56], Alu.subtract)

        u = crow[:, :, 10:14]
        nc.vector.tensor_tensor(u, crow[:, :, 56:60], delta_all[:, :, 0:4], Alu.add)
        u2 = crow[:, :, 15:19]
        nc.vector.tensor_tensor(u2, crow[:, :, 60:64], delta_all[:, :, 0:4], Alu.add)
        nc.scalar.activation(u, u, Act.Sigmoid)
        nc.scalar.activation(u2, u2, Act.Sigmoid)
        nc.vector.tensor_copy(crow[:, :, 14], delta_all[:, :, 4])
        nc.vector.tensor_copy(crow[:, :, 19], delta_all[:, :, 4])
        for c in range(5):
            nc.vector.tensor_tensor(crow[:, :, 10 + c], crow[:, :, 10 + c], cr3(30), Alu.mult)
            nc.vector.tensor_tensor(crow[:, :, 15 + c], crow[:, :, 15 + c], cr3(31), Alu.mult)

        # ------- S14: scatter -------
        sdep(nc.gpsimd.dma_scatter_add(
            canvas[:, :], crow[:, :, :], scir[:, :], 2048, 2048, EW))

    nc.compile()
    return nc


def _prep_consts():
    iotap1w = np.zeros((16, 504), np.float32)
    for b in range(8):
        for k in range(1000):
            iotap1w[k % 16, 63 * b + k // 16] = k + 1
    iotaw = np.zeros((16, 16), np.float32)
    for k in range(256):
        iotaw[k % 16, k // 16] = k
    iotaw2 = np.zeros((16, 32), np.float32)
    for k in range(512):
        iotaw2[k % 16, k // 16] = k
    return iotap1w, iotaw, iotaw2


def make_in_maps(inputs):
    logits = np.ascontiguousarray(inputs["logits"], np.float32)
    boxes = np.ascontiguousarray(inputs["boxes"], np.float32)
    features = np.ascontiguousarray(inputs["features"], np.float32)
    n_cores = 8
    bpc = logits.shape[0] // n_cores
    iotap1w, iotaw, iotaw2 = _prep_consts()
    bf = ml_dtypes.bfloat16
    wp1 = inputs["W_p1"].astype(bf); wp2 = inputs["W_p2"].astype(bf)
    wb1 = inputs["W_b1"].astype(bf); wb2 = inputs["W_b2"].astype(bf)
    whd = np.zeros((D, 8), np.float32)
    whd[:, 0:4] = inputs["W_b3"]; whd[:, 4:5] = inputs["W_s"]
    whd = whd.astype(bf)
    cvecT = np.zeros((D, 8), np.float32)
    cvecT[:, 0] = inputs["b_p1"]; cvecT[:, 1] = inputs["b_p2"]
    cvecT[:, 2] = inputs["b_b1"]; cvecT[:, 3] = inputs["b_b2"]
    cvecT[:, 4] = inputs["ln_g"]; cvecT[:, 5] = inputs["ln_b"]
    cvecT[0:4, 6] = inputs["b_b3"]; cvecT[0, 7] = inputs["b_s"][0]
    in_maps = []
    for c in range(n_cores):
        sl = slice(c * bpc, (c + 1) * bpc)
        bl = np.zeros((NROW, EW), np.float32)
        bl[:8000, 0:4] = boxes[sl].reshape(-1, 4)
        bl[:8000, 4:10] = logits[sl].reshape(-1, 6)
        fb = np.zeros((8001, D), bf)
        fb[:8000] = features[sl].reshape(-1, D).astype(bf)
        in_maps.append(dict(
            boxlog=bl, featbf=fb, wp1=wp1, wp2=wp2, wb1=wb1, wb2=wb2,
            whd=whd, cvecT=cvecT, iotap1w=iotap1w, iotaw=iotaw, iotaw2=iotaw2,
            identd=np.eye(128, dtype=np.float32)))
    return in_maps, bpc


def unpack(results, Bf, bpc):
    out = np.zeros((Bf, Q, 2, 5), np.float32)
    mask = np.zeros((Bf, Q, 2), bool)
    for c in range(len(results)):
        r = results[c]
        cvs = r["canvas"]; mk = r["bounce"]
        for b in range(bpc):
            rows = cvs[1001 * b:1001 * b + 1000, 10:20]
            out[c * bpc + b, :, 0, :] = rows[:, 0:5]
            out[c * bpc + b, :, 1, :] = rows[:, 5:10]
            mask[c * bpc + b, :, 0] = mk[1000 * b:1000 * b + 1000, 30] > 0.5
            mask[c * bpc + b, :, 1] = mk[1000 * b:1000 * b + 1000, 31] > 0.5
    return out, mask


def kernel(**inputs):
    if "nc" not in _CACHED:
        _CACHED["nc"] = build()
    nc = _CACHED["nc"]
    in_maps, bpc = make_in_maps(inputs)
    res = run_bass_kernel_spmd(nc, in_maps, core_ids=list(range(8)))
    return unpack(res.results, inputs["logits"].shape[0], bpc)


# revision 21
# speedup vs baseline: 1.4699x; 1.2629x over previous
"""Trainium2 Bass kernel for nn_AnatomicalRelationshipModule.

Data-parallel over 8 NeuronCores (8 images each). Per core:
  - dense per-box quantities + PE transposes to row-major planes
  - pairwise IoU>0.5 test (exact fp32 replica of reference math) with
    PE matmul OR-reduction against arytenoid flags
  - candidate mask, stream-compaction of candidate rows (sparse_gather)
  - MLP refinement only on gathered candidate rows (dma_gather, bf16 PE)
  - output assembled by dma_scatter_add onto a zeroed canvas
"""
import sys
import numpy as np

sys.path.insert(0, '/opt/trn_rl_repo')

import concourse.bass as bass
import concourse.bacc as bacc
import concourse.mybir as mybir
import concourse.tile as tile
from concourse import library_config
from concourse.bass import _add_dep_helper
from concourse.bass_utils import run_bass_kernel_spmd
from contextlib import ExitStack
import ml_dtypes

f32 = mybir.dt.float32
bf16 = mybir.dt.bfloat16
i16 = mybir.dt.int16
u32 = mybir.dt.uint32
Alu = mybir.AluOpType
Act = mybir.ActivationFunctionType

B_LOC = 8
Q = 1000
D = 256
THR = np.float32(0.84729785)   # fp32 boundary of sigmoid(x) > 0.7
CANDP = 256
NROW = 8064
EW = 64

_CACHED = {}


def build():
    nc = bacc.Bacc("TRN2", target_bir_lowering=False, debug=False)
    boxlog = nc.dram_tensor("boxlog", [NROW, EW], f32, kind="ExternalInput").ap()
    featbf = nc.dram_tensor("featbf", [8001, D], bf16, kind="ExternalInput").ap()
    wp1 = nc.dram_tensor("wp1", [D, D], bf16, kind="ExternalInput").ap()
    wp2 = nc.dram_tensor("wp2", [D, D], bf16, kind="ExternalInput").ap()
    wb1 = nc.dram_tensor("wb1", [D, D], bf16, kind="ExternalInput").ap()
    wb2 = nc.dram_tensor("wb2", [D, D], bf16, kind="ExternalInput").ap()
    whd = nc.dram_tensor("whd", [D, 8], bf16, kind="ExternalInput").ap()
    cvecT = nc.dram_tensor("cvecT", [D, 8], f32, kind="ExternalInput").ap()
    iotap1w = nc.dram_tensor("iotap1w", [16, 504], f32, kind="ExternalInput").ap()
    iotaw = nc.dram_tensor("iotaw", [16, 16], f32, kind="ExternalInput").ap()
    iotaw2 = nc.dram_tensor("iotaw2", [16, 32], f32, kind="ExternalInput").ap()
    identd = nc.dram_tensor("identd", [128, 128], f32, kind="ExternalInput").ap()

    canvas = nc.dram_tensor("canvas", [8192, EW], f32, kind="ExternalOutput").ap()
    bounce = nc.dram_tensor("bounce", [NROW, EW], f32, kind="ExternalOutput").ap()
    bounce2 = nc.dram_tensor("bounce2", [6, NROW], f32).ap()

    seq = [None, []]

    def sdep(binst):
        if seq[0] is not None:
            _add_dep_helper(binst.ins, seq[0].ins, True, "lib dep")
        seq[1].append(binst)
        return binst

    def sload(lib):
        b = nc.gpsimd.load_library(lib)
        for p in seq[1]:
            _add_dep_helper(b.ins, p.ins, True, "lib order")
        seq[0] = b
        seq[1] = []
        return b

    with tile.TileContext(nc) as tc, ExitStack() as ctx:
        pc = ctx.enter_context(tc.tile_pool(name="const", bufs=1))
        pd = ctx.enter_context(tc.tile_pool(name="dense", bufs=1))
        pw = ctx.enter_context(tc.tile_pool(name="work", bufs=2))
        psc = ctx.enter_context(tc.tile_pool(name="scratch", bufs=1))
        ps_ = ctx.enter_context(tc.tile_pool(name="small", bufs=2))
        pp = ctx.enter_context(tc.tile_pool(name="psum", bufs=1, space="PSUM"))
        pp2 = ctx.enter_context(tc.tile_pool(name="psum2", bufs=2, space="PSUM"))
        pb = ctx.enter_context(tc.tile_pool(name="big", bufs=1))

        # ------- S0: zero canvas -------
        idn = pc.tile([128, 128], f32)
        nc.sync.dma_start(idn[:], identd[:, :])
        zsb = pc.tile([128, 512], f32)
        nc.vector.memset(zsb[:], 0.0)
        for c in range(8):
            nc.sync.dma_start(canvas[1024 * c:1024 * (c + 1), :], zsb[:])

        # ------- S1: dense load -------
        # cols: 0 cx 1 cy 2 w 3 h 4..9 logits | 10 x1 11 x2 12 y1 13 y2 14 area
        #       15 lf 16 rf 17 la 18 ra 20 halfw 21 halfh 30 cl 31 cr
        dense = pd.tile([128, EW, EW], f32)
        nc.vector.memset(dense[:], 0.0)
        slot_rows = []
        for b in range(B_LOC):
            for jt in range(8):
                s = 8 * b + jt
                r0 = 1000 * b + 128 * jt
                n = 104 if jt == 7 else 128
                slot_rows.append((s, r0, n))
                nc.sync.dma_start(dense[0:n, s, :], boxlog[r0:r0 + n, :])

        # ------- S2: derived cols -------
        def dc3(c):
            return dense[:, :, c]
        nc.vector.tensor_scalar(dc3(20), dc3(2), 0.5, None, Alu.mult)
        nc.vector.tensor_scalar(dc3(21), dc3(3), 0.5, None, Alu.mult)
        nc.vector.tensor_tensor(dc3(10), dc3(0), dc3(20), Alu.subtract)
        nc.vector.tensor_tensor(dc3(11), dc3(0), dc3(20), Alu.add)
        nc.vector.tensor_tensor(dc3(12), dc3(1), dc3(21), Alu.subtract)
        nc.vector.tensor_tensor(dc3(13), dc3(1), dc3(21), Alu.add)
        nc.vector.tensor_tensor(dc3(14), dc3(2), dc3(3), Alu.mult)
        nc.vector.tensor_scalar(dc3(16), dc3(4), float(THR), None, Alu.is_gt)  # lf
        nc.vector.tensor_scalar(dc3(17), dc3(8), float(THR), None, Alu.is_gt)  # rf
        nc.vector.tensor_scalar(dc3(18), dc3(5), float(THR), None, Alu.is_gt)  # la
        nc.vector.tensor_scalar(dc3(19), dc3(9), float(THR), None, Alu.is_gt)  # ra
        nc.vector.tensor_tensor(dc3(15), dc3(18), dc3(19), Alu.max)  # aryor

        # ------- S3: transposes -> planes (0 x1,1 x2,2 y1,3 y2,4 area,5 lf,6 rf) -------
        plane = pb.tile([6, NROW], f32)
        nc.vector.memset(plane[0:6, 8000:NROW], 0.0)
        for b in range(B_LOC):
            for half in range(2):
                tp = pp.tile([7, 512], f32, tag="tp_any")
                width = 0
                for ji, jt in enumerate(range(4 * half, 4 * half + 4)):
                    s = 8 * b + jt
                    n = 104 if jt == 7 else 128
                    nc.tensor.transpose(
                        tp[0:6, 128 * ji:128 * ji + n],
                        dense[0:n, s, 10:16], idn[0:n, 0:n])
                    width += n
                c0 = 1000 * b + 512 * half
                nc.scalar.copy(plane[0:6, c0:c0 + width], tp[0:6, 0:width])

        # ------- S4b: ary compaction -------
        nc.sync.dma_start(bounce2[:, :], plane[:, :])
        iopA = pc.tile([16, 504], f32, tag="iopA")
        nc.sync.dma_start(iopA[:], iotap1w[:, :])
        iowA = pc.tile([16, 32], f32, tag="iowA")
        nc.sync.dma_start(iowA[:], iotaw2[:, :])
        awj = ps_.tile([16, 504], f32, tag="awj")
        for b in range(B_LOC):
            nc.sync.dma_start(
                awj[:, 63 * b:63 * b + 63],
                bounce2[5, 1000 * b:1000 * b + 1008].rearrange("(c p) -> p c", p=16))
        ivA = ps_.tile([16, 504], f32, tag="ivA")
        nc.vector.tensor_tensor(ivA[:, :], awj[:, :], iopA[:, :], Alu.mult)
        nc.vector.tensor_scalar(ivA[:, :], ivA[:, :], 1.0, None, Alu.subtract)
        sgoA = ps_.tile([16, 256], f32, tag="sgoA")
        nc.vector.memset(sgoA[:], -1.0)
        cntsA = ps_.tile([1, 8], u32, tag="cntsA")
        nc.gpsimd.memset(cntsA[:], 0)
        sload(library_config.sparse_gather)
        for b in range(B_LOC):
            sdep(nc.gpsimd.sparse_gather(
                sgoA[:, 32 * b:32 * b + 32], ivA[:, 63 * b:63 * b + 63],
                num_found=cntsA[0:1, b:b + 1]))
        cntfA = ps_.tile([1, 8], f32, tag="cntfA")
        nc.vector.tensor_copy(cntfA[:], cntsA[:])
        sload(library_config.mlp)
        cnbA = ps_.tile([16, 8], f32, tag="cnbA")
        sdep(nc.gpsimd.partition_broadcast(cnbA[:], cntfA[:]))
        valA = ps_.tile([16, 256], f32, tag="valA")
        gaj = ps_.tile([16, 256], f32, tag="gaj")
        for b in range(B_LOC):
            sl = (slice(0, 16), slice(32 * b, 32 * b + 32))
            nc.vector.tensor_scalar(valA[sl], iowA[:, :], cnbA[:, b:b + 1], None, Alu.is_lt)
            nc.vector.tensor_scalar(gaj[sl], sgoA[sl], float(1000 * b - 8000), None, Alu.add)
            nc.vector.tensor_tensor(gaj[sl], gaj[sl], valA[sl], Alu.mult)
            nc.vector.tensor_scalar(gaj[sl], gaj[sl], 8000.0, None, Alu.add)
        gaj16 = ps_.tile([16, 256], i16, tag="gaj16")
        nc.vector.tensor_copy(gaj16[:], gaj[:])
        gajr = ps_.tile([128, 256], i16, tag="gajr")
        for g in range(8):
            nc.sync.dma_start(gajr[16 * g:16 * g + 16, :], gaj16[:, :])
        jrow = pd.tile([128, 32, EW], f32)
        nc.vector.memset(jrow[:], 0.0)
        for b in range(B_LOC):
            sdep(nc.gpsimd.dma_gather(
                jrow[:, 4 * b:4 * b + 4, :], boxlog[:, :],
                gajr[:, 32 * b:32 * b + 32], 512, 512, EW))
        # derived cols on gathered j rows
        def jc3(c):
            return jrow[:, :, c]
        nc.vector.tensor_scalar(jc3(20), jc3(2), 0.5, None, Alu.mult)
        nc.vector.tensor_scalar(jc3(21), jc3(3), 0.5, None, Alu.mult)
        nc.vector.tensor_tensor(jc3(10), jc3(0), jc3(20), Alu.subtract)
        nc.vector.tensor_tensor(jc3(11), jc3(0), jc3(20), Alu.add)
        nc.vector.tensor_tensor(jc3(12), jc3(1), jc3(21), Alu.subtract)
        nc.vector.tensor_tensor(jc3(13), jc3(1), jc3(21), Alu.add)
        nc.vector.tensor_tensor(jc3(14), jc3(2), jc3(3), Alu.mult)
        nc.vector.tensor_scalar(jc3(18), jc3(5), float(THR), None, Alu.is_gt)
        nc.vector.tensor_scalar(jc3(19), jc3(9), float(THR), None, Alu.is_gt)
        awf = pd.tile([128, 32, 2], bf16)
        nc.vector.tensor_copy(awf[:, :, 0], jc3(18))
        nc.vector.tensor_copy(awf[:, :, 1], jc3(19))
        sload(library_config.standard)

        # ------- S5: pairwise + PE reduce -------
        for b in range(B_LOC):
            bcs = []
            for q in range(5):
                t = psc.tile([128, 1008], f32, tag=f"bc{q}")
                nc.sync.dma_start(
                    t[:, :], bounce2[q, 1000 * b:1000 * b + 1008].partition_broadcast(128))
                bcs.append(t)
            bx1, bx2, by1, by2, bar = bcs
            pr = pp.tile([2, 1008], f32, tag="pred")
            for jt in range(4):
                s = 4 * b + jt
                n = 128
                j = lambda c: jrow[0:n, s, c:c + 1]
                a = psc.tile([128, 1008], f32, tag="sa")
                bq = psc.tile([128, 1008], f32, tag="sb")
                cq = psc.tile([128, 1008], f32, tag="sc")
                dq = psc.tile([128, 1008], f32, tag="sd")
                nc.vector.tensor_scalar(a[0:n, :], bx1[0:n, :], j(10), None, Alu.max)
                nc.vector.tensor_scalar(bq[0:n, :], bx2[0:n, :], j(11), None, Alu.min)
                nc.vector.tensor_tensor(bq[0:n, :], bq[0:n, :], a[0:n, :], Alu.subtract)
                nc.scalar.activation(a[0:n, :], bq[0:n, :], Act.Relu)
                nc.vector.tensor_scalar(cq[0:n, :], by1[0:n, :], j(12), None, Alu.max)
                nc.vector.tensor_scalar(dq[0:n, :], by2[0:n, :], j(13), None, Alu.min)
                nc.vector.tensor_tensor(dq[0:n, :], dq[0:n, :], cq[0:n, :], Alu.subtract)
                nc.scalar.activation(cq[0:n, :], dq[0:n, :], Act.Relu)
                nc.vector.tensor_tensor(a[0:n, :], a[0:n, :], cq[0:n, :], Alu.mult)  # inter
                nc.scalar.activation(cq[0:n, :], bar[0:n, :], Act.Identity, bias=j(14))  # S
                sdep(nc.gpsimd.tensor_tensor(cq[0:n, :], cq[0:n, :], a[0:n, :], Alu.subtract))  # U
                nc.scalar.mul(a[0:n, :], a[0:n, :], 2.0)  # 2*inter
                r01 = pw.tile([128, 1008], bf16, tag="r01")
                nc.vector.tensor_tensor(r01[0:n, :], cq[0:n, :], a[0:n, :], Alu.is_lt)
                nc.tensor.matmul(pr[:, 0:512], awf[0:n, s, :], r01[0:n, 0:512],
                                 start=(jt == 0), stop=(jt == 3))
                nc.tensor.matmul(pr[:, 512:1008], awf[0:n, s, :], r01[0:n, 512:1008],
                                 start=(jt == 0), stop=(jt == 3), skip_group_check=True)
            mgt = ps_.tile([2, 1008], f32, tag="mgt")
            nc.scalar.activation(mgt[0:2, :], pr[0:2, :], Act.Sign)
            pt = pp.tile([128, 16], f32, tag="tp_any")
            nc.vector.memset(pt[:], 0.0)
            for jt in range(8):
                n = 104 if jt == 7 else 128
                nc.tensor.transpose(pt[0:n, 2 * jt:2 * jt + 2],
                                    mgt[0:2, 128 * jt:128 * jt + n], idn[0:2, 0:2])
            nc.vector.tensor_copy(
                dense[:, 8 * b:8 * b + 8, 28:30],
                pt[:, :].rearrange("p (s c) -> p s c", s=8))

        # ------- S6: cand mask in dense cols: 28 hl 29 hr -> 30 cl 31 cr 32 candor ---
        nc.vector.tensor_scalar(dc3(33), dc3(28), -1.0, 1.0, Alu.mult, Alu.add)
        nc.vector.tensor_tensor(dc3(30), dc3(16), dc3(33), Alu.mult)
        nc.vector.tensor_scalar(dc3(33), dc3(29), -1.0, 1.0, Alu.mult, Alu.add)
        nc.vector.tensor_tensor(dc3(31), dc3(17), dc3(33), Alu.mult)
        nc.vector.tensor_tensor(dc3(32), dc3(30), dc3(31), Alu.max)

        # ------- S8: bounce write -------
        nc.sync.dma_start(bounce[8000:NROW, :], zsb[0:64, 0:64])
        for (s, r0, n) in slot_rows:
            nc.sync.dma_start(bounce[r0:r0 + n, :], dense[0:n, s, :])

        # ------- S9: candidate indices -------
        cwl = ps_.tile([16, 504], f32, tag="cwl")
        for b in range(B_LOC):
            nc.sync.dma_start(
                cwl[:, 63 * b:63 * b + 63],
                bounce[1000 * b:1000 * b + 1008, 32:33].rearrange("(c p) o -> (o p) c", p=16))
        iop = pc.tile([16, 504], f32)
        nc.sync.dma_start(iop[:], iotap1w[:, :])
        iow = pc.tile([16, 16], f32)
        nc.sync.dma_start(iow[:], iotaw[:, :])
        iv = ps_.tile([16, 504], f32, tag="iv")
        nc.vector.tensor_tensor(iv[:, :], cwl[:, :], iop[:, :], Alu.mult)
        nc.vector.tensor_scalar(iv[:, :], iv[:, :], 1.0, None, Alu.subtract)

        sgo = ps_.tile([16, 128], f32, tag="sgo")
        nc.vector.memset(sgo[:], -1.0)
        cnts = ps_.tile([1, 8], u32, tag="cnts")
        nc.gpsimd.memset(cnts[:], 0)
        sload(library_config.sparse_gather)
        for b in range(B_LOC):
            sdep(nc.gpsimd.sparse_gather(
                sgo[:, 16 * b:16 * b + 16], iv[:, 63 * b:63 * b + 63],
                num_found=cnts[0:1, b:b + 1]))

        cntf = ps_.tile([1, 8], f32, tag="cntf")
        nc.vector.tensor_copy(cntf[:], cnts[:])
        sload(library_config.mlp)
        cnb = ps_.tile([16, 8], f32, tag="cnb")
        sdep(nc.gpsimd.partition_broadcast(cnb[:], cntf[:]))

        val = ps_.tile([16, 128], f32, tag="val")
        gfi = ps_.tile([16, 128], f32, tag="gfi")
        sci = ps_.tile([16, 128], f32, tag="sci")
        for b in range(B_LOC):
            sl = (slice(0, 16), slice(16 * b, 16 * b + 16))
            nc.vector.tensor_scalar(val[sl], iow[:, :], cnb[:, b:b + 1], None, Alu.is_lt)
            nc.vector.tensor_scalar(gfi[sl], sgo[sl], float(1000 * b - 8000), None, Alu.add)
            nc.vector.tensor_tensor(gfi[sl], gfi[sl], val[sl], Alu.mult)
            nc.vector.tensor_scalar(gfi[sl], gfi[sl], 8000.0, None, Alu.add)
            nc.vector.tensor_scalar(sci[sl], sgo[sl], 1000.0, None, Alu.subtract)
            nc.vector.tensor_tensor(sci[sl], sci[sl], val[sl], Alu.mult)
            nc.vector.tensor_scalar(sci[sl], sci[sl], float(1000 + 1001 * b), None, Alu.add)
        gfi16 = ps_.tile([16, 128], i16, tag="gfi16")
        sci16 = ps_.tile([16, 128], i16, tag="sci16")
        nc.vector.tensor_copy(gfi16[:], gfi[:])
        nc.vector.tensor_copy(sci16[:], sci[:])
        gfir = ps_.tile([128, 128], i16, tag="gfir")
        scir = ps_.tile([128, 128], i16, tag="scir")
        for g in range(8):
            nc.sync.dma_start(gfir[16 * g:16 * g + 16, :], gfi16[:, :])
            nc.sync.dma_start(scir[16 * g:16 * g + 16, :], sci16[:, :])

        # ------- S10: gathers -------
        frhs_t = []
        crow = pb.tile([128, 16, EW], f32)
        for b in range(B_LOC):
            ft = pd.tile([128, 2, 256], bf16, tag=f"frhs{b}")
            sdep(nc.gpsimd.dma_gather(
                ft[:, :, :], featbf[:, :],
                gfir[:, 16 * b:16 * b + 16], CANDP, CANDP, D, transpose=True))
            frhs_t.append(ft)
            sdep(nc.gpsimd.dma_gather(
                crow[:, 2 * b:2 * b + 2, :], bounce[:, :],
                gfir[:, 16 * b:16 * b + 16], CANDP, CANDP, EW))

        # ------- S11: MLP -------
        wsb = {}
        for nm, ap_ in [("wp1", wp1), ("wp2", wp2), ("wb1", wb1), ("wb2", wb2)]:
            t = pc.tile([128, 2, D], bf16, tag=f"w_{nm}")
            nc.sync.dma_start(t[:, 0, :], ap_[0:128, :])
            nc.sync.dma_start(t[:, 1, :], ap_[128:256, :])
            wsb[nm] = t
        whd_sb = pc.tile([128, 2, 8], bf16)
        nc.sync.dma_start(whd_sb[:, 0, :], whd[0:128, :])
        nc.sync.dma_start(whd_sb[:, 1, :], whd[128:256, :])
        # cvecT rows: per-channel cols: 0 bp1 1 bp2 2 bb1 3 bb2 4 lng 5 lnb 6 bhead 7 bs0
        cv = pc.tile([128, 2, 8], f32)
        nc.sync.dma_start(cv[:, 0, :], cvecT[0:128, :])
        nc.sync.dma_start(cv[:, 1, :], cvecT[128:256, :])
        ones_bf = pc.tile([128, 1], bf16)
        nc.vector.memset(ones_bf[:], 1.0)

        y1a = pb.tile([128, 2, 2048], bf16)
        stats_y = ps_.tile([8, 256], f32, tag="stats_y")
        stats_q = ps_.tile([8, 256], f32, tag="stats_q")
        for b in range(B_LOC):
            cols = slice(256 * b, 256 * b + 256)
            ysq = ps_.tile([128, 2, 256], bf16, tag="ysq")
            for m in range(2):
                psm = pp2.tile([128, 256], f32, tag="mlp")
                for k in range(2):
                    nc.tensor.matmul(psm[:, :], wsb["wp1"][:, k, 128 * m:128 * m + 128],
                                     frhs_t[b][:, k, :], start=(k == 0), stop=(k == 1))
                nc.scalar.activation(y1a[:, m, cols], psm[:, :], Act.Identity, bias=cv[:, m, 0:1])
                nc.scalar.activation(ysq[:, m, :], psm[:, :], Act.Square, bias=cv[:, m, 0:1])
            st0 = pp.tile([1, 256], f32, tag="stA")
            st1 = pp.tile([1, 256], f32, tag="stB")
            for m in range(2):
                nc.tensor.matmul(st0[:, :], ones_bf[:, :], y1a[:, m, cols],
                                 start=(m == 0), stop=(m == 1))
            for m in range(2):
                nc.tensor.matmul(st1[:, :], ones_bf[:, :], ysq[:, m, :],
                                 start=(m == 0), stop=(m == 1))
            ev0 = ps_.tile([1, 256], f32, tag="ev0")
            ev1 = ps_.tile([1, 256], f32, tag="ev1")
            nc.vector.tensor_copy(ev0[:, :], st0[:, :])
            nc.vector.tensor_copy(ev1[:, :], st1[:, :])
            nc.sync.dma_start(stats_y[b:b + 1, :], ev0[:, :])
            nc.sync.dma_start(stats_q[b:b + 1, :], ev1[:, :])

        s1 = ps_.tile([8, 256], f32, tag="s1")
        s1q = ps_.tile([8, 256], f32, tag="s1q")
        nc.vector.tensor_scalar(s1[:, :], stats_y[:, :], 1.0 / 256.0, None, Alu.mult)
        nc.vector.tensor_scalar(s1q[:, :], stats_q[:, :], 1.0 / 256.0, None, Alu.mult)
        var = ps_.tile([8, 256], f32, tag="var")
        nc.vector.tensor_tensor(var[:, :], s1[0:8, :], s1[0:8, :], Alu.mult)
        nc.vector.tensor_tensor(var[:, :], s1q[:, :], var[:, :], Alu.subtract)
        nc.vector.tensor_scalar(var[:, :], var[:, :], 1e-5, None, Alu.add)
        rec = ps_.tile([8, 256], f32, tag="rec")
        nc.vector.reciprocal(rec[:, :], var[:, :])
        rstd = ps_.tile([8, 256], f32, tag="rstd")
        nc.scalar.activation(rstd[:, :], rec[:, :], Act.Sqrt)
        lnB = ps_.tile([8, 256], f32, tag="lnB")
        nc.vector.tensor_tensor(lnB[:, :], s1[0:8, :], rstd[:, :], Alu.mult)
        nc.vector.tensor_scalar(lnB[:, :], lnB[:, :], -1.0, None, Alu.mult)

        bounce3 = nc.dram_tensor("bounce3", [16, D], f32).ap()
        nc.sync.dma_start(bounce3[0:8, :], rstd[:, :])
        nc.sync.dma_start(bounce3[8:16, :], lnB[:, :])
        delta_all = pb.tile([128, 16, 8], f32)
        for b in range(B_LOC):
            cols = slice(256 * b, 256 * b + 256)
            Ab = ps_.tile([128, 256], f32, tag="Ab")
            Bb = ps_.tile([128, 256], f32, tag="Bb")
            nc.sync.dma_start(Ab[:, :], bounce3[b, :].partition_broadcast(128))
            nc.sync.dma_start(Bb[:, :], bounce3[8 + b, :].partition_broadcast(128))
            h1t = ps_.tile([128, 2, 256], bf16, tag="h1t")
            for m in range(2):
                t1 = ps_.tile([128, 256], f32, tag="t1")
                nc.vector.tensor_tensor(t1[:, :], y1a[:, m, cols], Ab[:, :], Alu.mult)
                nc.vector.tensor_tensor(t1[:, :], t1[:, :], Bb[:, :], Alu.add)
                nc.vector.tensor_scalar(t1[:, :], t1[:, :], cv[:, m, 4:5], cv[:, m, 5:6],
                                        Alu.mult, Alu.add)
                nc.scalar.activation(h1t[:, m, :], t1[:, :], Act.Relu)
            aft = ps_.tile([128, 2, 256], bf16, tag="aft")
            for m in range(2):
                psm = pp2.tile([128, 256], f32, tag="mlp")
                for k in range(2):
                    nc.tensor.matmul(psm[:, :], wsb["wp2"][:, k, 128 * m:128 * m + 128],
                                     h1t[:, k, :], start=(k == 0), stop=(k == 1))
                nc.scalar.activation(aft[:, m, :], psm[:, :], Act.Identity, bias=cv[:, m, 1:2])
            m1t = ps_.tile([128, 2, 256], bf16, tag="m1t")
            for m in range(2):
                psm = pp2.tile([128, 256], f32, tag="mlp")
                for k in range(2):
                    nc.tensor.matmul(psm[:, :], wsb["wb1"][:, k, 128 * m:128 * m + 128],
                                     aft[:, k, :], start=(k == 0), stop=(k == 1))
                nc.scalar.activation(m1t[:, m, :], psm[:, :], Act.Relu, bias=cv[:, m, 2:3])
            m2t = ps_.tile([128, 2, 256], bf16, tag="m2t")
            for m in range(2):
                psm = pp2.tile([128, 256], f32, tag="mlp")
                for k in range(2):
                    nc.tensor.matmul(psm[:, :], wsb["wb2"][:, k, 128 * m:128 * m + 128],
                                     m1t[:, k, :], start=(k == 0), stop=(k == 1))
                nc.scalar.activation(m2t[:, m, :], psm[:, :], Act.Relu, bias=cv[:, m, 3:4])
            hd1 = pp.tile([4, 256], f32, tag="stA")
            hd2 = pp.tile([1, 256], f32, tag="stB")
            for k in range(2):
                nc.tensor.matmul(hd1[:, :], whd_sb[:, k, 0:4], m2t[:, k, :],
                                 start=(k == 0), stop=(k == 1))
            for k in range(2):
                nc.tensor.matmul(hd2[:, :], whd_sb[:, k, 4:5], aft[:, k, :],
                                 start=(k == 0), stop=(k == 1))
            hs1 = ps_.tile([4, 256], f32, tag="hs1")
            hs2 = ps_.tile([1, 256], f32, tag="hs2")
            nc.scalar.activation(hs1[:, :], hd1[:, :], Act.Identity, bias=cv[0:4, 0, 6:7])
            nc.scalar.activation(hs2[:, :], hd2[:, :], Act.Identity, bias=cv[0:1, 0, 7:8])
            tph = pp.tile([128, 2, 5], f32, tag="tp_any")
            for half in range(2):
                nc.tensor.transpose(tph[:, half, 0:4], hs1[0:4, 128 * half:128 * half + 128], idn[0:4, 0:4])
                nc.tensor.transpose(tph[:, half, 4:5], hs2[0:1, 128 * half:128 * half + 128], idn[0:1, 0:1])
            nc.vector.tensor_copy(delta_all[:, 2 * b:2 * b + 2, 0:5], tph[:, :, :])

        # ------- S13: value path on candidate rows -------
        cr3 = lambda c: crow[:, :, c]
        nc.vector.tensor_scalar(cr3(33), cr3(3), 0.5, None, Alu.mult)
        nc.vector.tensor_tensor(cr3(32), cr3(1), cr3(33), Alu.add)
        nc.vector.tensor_scalar(cr3(34), cr3(2), 0.8, None, Alu.mult)
        nc.vector.tensor_scalar(cr3(35), cr3(3), 0.8, None, Alu.mult)
        nc.vector.tensor_scalar(cr3(37), cr3(2), 0.1, None, Alu.mult)
        nc.vector.tensor_tensor(cr3(36), cr3(0), cr3(37), Alu.add)
        srcs = [0, 32, 34, 35, 36, 32, 34, 35]
        for i, c in enumerate(srcs):
            nc.vector.tensor_scalar(cr3(40 + i), cr3(c), 1e-5, 1.0, Alu.max, Alu.min)
            nc.vector.tensor_scalar(cr3(48 + i), cr3(c), -1.0, 1.0, Alu.mult, Alu.add)
            nc.vector.tensor_scalar(cr3(48 + i), cr3(48 + i), 1e-5, 1.0, Alu.max, Alu.min)
        lg = crow[:, :, 40:56]
        nc.scalar.activation(lg, lg, Act.Ln)
        nc.vector.tensor_tensor(
            crow[:, :, 56:64], crow[:, :, 40:48],
            crow[:, :, 48:56], Alu.subtract)

        u = crow[:, :, 10:14]
        nc.vector.tensor_tensor(u, crow[:, :, 56:60], delta_all[:, :, 0:4], Alu.add)
        u2 = crow[:, :, 15:19]
        nc.vector.tensor_tensor(u2, crow[:, :, 60:64], delta_all[:, :, 0:4], Alu.add)
        nc.scalar.activation(u, u, Act.Sigmoid)
        nc.scalar.activation(u2, u2, Act.Sigmoid)
        nc.vector.tensor_copy(crow[:, :, 14], delta_all[:, :, 4])
        nc.vector.tensor_copy(crow[:, :, 19], delta_all[:, :, 4])
        for c in range(5):
            nc.vector.tensor_tensor(crow[:, :, 10 + c], crow[:, :, 10 + c], cr3(30), Alu.mult)
            nc.vector.tensor_tensor(crow[:, :, 15 + c], crow[:, :, 15 + c], cr3(31), Alu.mult)

        # ------- S14: scatter -------
        sdep(nc.gpsimd.dma_scatter_add(
            canvas[:, :], crow[:, :, :], scir[:, :], 2048, 2048, EW))

    nc.compile()
    return nc


def _prep_consts():
    iotap1w = np.zeros((16, 504), np.float32)
    for b in range(8):
        for k in range(1000):
            iotap1w[k % 16, 63 * b + k // 16] = k + 1
    iotaw = np.zeros((16, 16), np.float32)
    for k in range(256):
        iotaw[k % 16, k // 16] = k
    iotaw2 = np.zeros((16, 32), np.float32)
    for k in range(512):
        iotaw2[k % 16, k // 16] = k
    return iotap1w, iotaw, iotaw2


def make_in_maps(inputs):
    logits = np.ascontiguousarray(inputs["logits"], np.float32)
    boxes = np.ascontiguousarray(inputs["boxes"], np.float32)
    features = np.ascontiguousarray(inputs["features"], np.float32)
    n_cores = 8
    bpc = logits.shape[0] // n_cores
    iotap1w, iotaw, iotaw2 = _prep_consts()
    bf = ml_dtypes.bfloat16
    wp1 = inputs["W_p1"].astype(bf); wp2 = inputs["W_p2"].astype(bf)
    wb1 = inputs["W_b1"].astype(bf); wb2 = inputs["W_b2"].astype(bf)
    whd = np.zeros((D, 8), np.float32)
    whd[:, 0:4] = inputs["W_b3"]; whd[:, 4:5] = inputs["W_s"]
    whd = whd.astype(bf)
    cvecT = np.zeros((D, 8), np.float32)
    cvecT[:, 0] = inputs["b_p1"]; cvecT[:, 1] = inputs["b_p2"]
    cvecT[:, 2] = inputs["b_b1"]; cvecT[:, 3] = inputs["b_b2"]
    cvecT[:, 4] = inputs["ln_g"]; cvecT[:, 5] = inputs["ln_b"]
    cvecT[0:4, 6] = inputs["b_b3"]; cvecT[0, 7] = inputs["b_s"][0]
    in_maps = []
    for c in range(n_cores):
        sl = slice(c * bpc, (c + 1) * bpc)
        bl = np.zeros((NROW, EW), np.float32)
        bl[:8000, 0:4] = boxes[sl].reshape(-1, 4)
        bl[:8000, 4:10] = logits[sl].reshape(-1, 6)
        fb = np.zeros((8001, D), bf)
        fb[:8000] = features[sl].reshape(-1, D).astype(bf)
        in_maps.append(dict(
            boxlog=bl, featbf=fb, wp1=wp1, wp2=wp2, wb1=wb1, wb2=wb2,
            whd=whd, cvecT=cvecT, iotap1w=iotap1w, iotaw=iotaw, iotaw2=iotaw2,
            identd=np.eye(128, dtype=np.float32)))
    return in_maps, bpc


def unpack(results, Bf, bpc):
    out = np.zeros((Bf, Q, 2, 5), np.float32)
    mask = np.zeros((Bf, Q, 2), bool)
    for c in range(len(results)):
        r = results[c]
        cvs = r["canvas"]; mk = r["bounce"]
        for b in range(bpc):
            rows = cvs[1001 * b:1001 * b + 1000, 10:20]
            out[c * bpc + b, :, 0, :] = rows[:, 0:5]
            out[c * bpc + b, :, 1, :] = rows[:, 5:10]
            mask[c * bpc + b, :, 0] = mk[1000 * b:1000 * b + 1000, 30] > 0.5
            mask[c * bpc + b, :, 1] = mk[1000 * b:1000 * b + 1000, 31] > 0.5
    return out, mask


def kernel(**inputs):
    if "nc" not in _CACHED:
        _CACHED["nc"] = build()
    nc = _CACHED["nc"]
    in_maps, bpc = make_in_maps(inputs)
    res = run_bass_kernel_spmd(nc, in_maps, core_ids=list(range(8)))
    return unpack(res.results, inputs["logits"].shape[0], bpc)
